# revision 15
# baseline (speedup 1.0000x reference)
"""Trainium2 Bass kernel for sparse (rns-masked) attention.

Problem (reference.py): x:[4,1024,1024] f32; qkv = x@W_attn+b; 16 heads x 64;
w = q k^T / 8; mask m[b,i,j] = (j in rns[b,i]) AND (i in rns[b,j]);
softmax(w*m - 1e9*(1-m)); a = p @ v; out = a @ W_proj + b_proj.

Sharding: 8 cores = batch (4) x head-group (2 groups of 8 heads).
Each core computes a partial output projection for its 8 heads; host sums the
two partials per batch and adds b_proj.

On-device pipeline per core (all matmuls bf16 with f32 PSUM accumulation):
  - A[i,j] = [j in rns(i)] decoded on host (the GpSimd scatter instructions
    fail codegen on this toolchain: "ISA wrong length"); shipped as a bf16
    input. AT via PE transpose; m = A * AT elementwise (m is symmetric).
  - qkT[c,t] (q,k transposed) via lhsT=W_qk, rhs=xT.  v[t,c] via lhsT=xT.
    Biases folded in as rank-1 (K=1) matmul accumulation when nonzero.
  - Scores computed transposed per head: sT[j,i] = k_j . q_i; p = m*exp(sT/8)
    (no row-max needed: |scores| <= ~4, exp cannot overflow).
  - a[i,d] = sum_j pT[j,i] vext[j,d] with vext = [v | 1] so column 64
    accumulates the softmax denominator.
  - Empty rows (denominator 0 -> reference softmax is uniform): replace with
    SV/1024 where SV = sum_j v[j]; done via per-partition flag + predicated
    copy from a PE-replicated SV/1024 tile.
  - a transposed per head-pair via PE; out_partial = aT @ W_proj slice.
"""

import os
import sys

import numpy as np

try:
    import concourse.bass as bass
except ImportError:  # harness containers keep the repo at /opt/trn_rl_repo
    sys.path.insert(0, "/opt/trn_rl_repo")
    import concourse.bass as bass

import ml_dtypes

import concourse.mybir as mybir
import concourse.tile as tile
from concourse import bacc
from concourse.bass_utils import run_bass_kernel_spmd
from concourse.masks import make_identity

BF16 = mybir.dt.bfloat16
F32 = mybir.dt.float32
NPBF = ml_dtypes.bfloat16

P = 128
DL = 1024  # sequence length
E = 1024  # embed dim
DH = 64  # head dim
HPC = 8  # heads per core
KT = 8  # contraction tiles over E
IT = 8  # i tiles (queries)
JT = 8  # j tiles (keys)

LAST_RESULT = None  # stashed BassKernelResults for test harness introspection


def build_body(tc, ins, outs, use_bias):
    """Emit one core's program. ins/outs: dicts of DRAM APs."""
    nc = tc.nc
    AF = mybir.ActivationFunctionType

    with (
        tc.tile_pool(name="persist", bufs=1) as pp,
        tc.tile_pool(name="pT", bufs=2) as pT_pool,
        tc.tile_pool(name="apair", bufs=2) as apair_pool,
        tc.tile_pool(name="outst", bufs=2) as outst_pool,
        tc.tile_pool(name="small", bufs=8) as small_pool,
        tc.tile_pool(name="ps_mm", bufs=3, space="PSUM") as ps_mm,
        tc.tile_pool(name="ps_pv", bufs=2, space="PSUM") as ps_pv,
        tc.tile_pool(name="ps_tr", bufs=1, space="PSUM") as ps_tr,
        tc.tile_pool(name="ps_sv", bufs=1, space="PSUM") as ps_sv,
        tc.tile_pool(name="ps_svd", bufs=1, space="PSUM") as ps_svd,
    ):
        # ---- persistent SBUF tensors
        xT_sb = pp.tile([P, KT, DL], BF16, tag="xT")
        wqk_sb = pp.tile([P, KT, 1024], BF16, tag="wqk")
        wv_sb = pp.tile([P, KT, 512], BF16, tag="wv")
        wpj_sb = pp.tile([P, 4, DL], BF16, tag="wpj")
        A_sb = pp.tile([P, IT, DL], BF16, tag="A")
        AT_sb = pp.tile([P, JT, DL], BF16, tag="AT")
        m_sb = pp.tile([P, JT, DL], BF16, tag="m")
        qkT_sb = pp.tile([P, 8, DL], BF16, tag="qkT")
        vext_sb = pp.tile([P, JT, 520], BF16, tag="vext")
        aT_sb = pp.tile([P, 4, DL], BF16, tag="aT")
        ident = pp.tile([P, P], BF16, tag="ident")
        ones_row = pp.tile([1, 512], BF16, tag="ones_row")
        ones_col = pp.tile([P, 1], BF16, tag="ones_col")
        inv_row = pp.tile([1, P], BF16, tag="inv_row")
        svh_sb = pp.tile([1, HPC, DH], BF16, tag="svh")
        if use_bias:
            baqk_sb = pp.tile([1, 1024], BF16, tag="baqk")
            bav_sb = pp.tile([1, 512], BF16, tag="bav")

        # ---- input DMAs
        nc.sync.dma_start(xT_sb[:], ins["xT"].rearrange("(ko ki) t -> ki ko t", ki=P))
        nc.sync.dma_start(wqk_sb[:], ins["wqk"].rearrange("(ko ki) c -> ki ko c", ki=P))
        nc.sync.dma_start(wv_sb[:], ins["wv"].rearrange("(ko ki) c -> ki ko c", ki=P))
        nc.sync.dma_start(wpj_sb[:], ins["wp"].rearrange("(ko ki) j -> ki ko j", ki=P))
        nc.sync.dma_start(A_sb[:], ins["Ab"].rearrange("(io ii) j -> ii io j", ii=P))
        if use_bias:
            nc.sync.dma_start(baqk_sb[:], ins["baqk"][:])
            nc.sync.dma_start(bav_sb[:], ins["bav"][:])

        # ---- constants
        make_identity(nc, ident[:])
        nc.gpsimd.memset(ones_row[:], 1.0)
        nc.gpsimd.memset(ones_col[:], 1.0)
        nc.gpsimd.memset(inv_row[:], 1.0 / DL)
        # denominator columns of vext ([v | 1] per head)
        vext_h = vext_sb.rearrange("p a (h c) -> p a h c", c=65)
        nc.gpsimd.memset(vext_h[:, :, :, 64], 1.0)

        # ---- mask: AT, m from host-decoded A
        for it in range(IT):
            for jt in range(JT):
                pst = ps_tr.tile([P, P], BF16, tag="tr")
                nc.tensor.transpose(pst[:], A_sb[:, it, bass.ts(jt, P)], ident[:])
                nc.vector.tensor_copy(AT_sb[:, jt, bass.ts(it, P)], pst[:])
        for jt in range(JT):
            nc.vector.tensor_mul(m_sb[:, jt, :], A_sb[:, jt, :], AT_sb[:, jt, :])

        # ---- qkT[c, t] = (x @ Wqk).T : lhsT = Wqk (e, c), rhs = xT (e, t)
        for mt in range(8):
            for nt in range(2):
                ps = ps_mm.tile([P, 512], F32, tag="mm")
                for kt in range(KT):
                    nc.tensor.matmul(
                        ps[:], wqk_sb[:, kt, bass.ts(mt, P)],
                        xT_sb[:, kt, bass.ts(nt, 512)],
                        start=(kt == 0), stop=(kt == KT - 1 and not use_bias),
                    )
                if use_bias:
                    nc.tensor.matmul(
                        ps[:], baqk_sb[0:1, bass.ts(mt, P)],
                        ones_row[0:1, :], start=False, stop=True,
                    )
                nc.scalar.copy(qkT_sb[:, mt, bass.ts(nt, 512)], ps[:])

        # ---- v[t, c] : lhsT = xT (e, t), rhs = Wv (e, c); strided into vext
        for mt in range(8):
            ps = ps_mm.tile([P, 512], F32, tag="mm")
            for kt in range(KT):
                nc.tensor.matmul(
                    ps[:], xT_sb[:, kt, bass.ts(mt, P)], wv_sb[:, kt, :],
                    start=(kt == 0), stop=(kt == KT - 1 and not use_bias),
                )
            if use_bias:
                nc.tensor.matmul(
                    ps[:], ones_row[0:1, 0:P], bav_sb[0:1, :],
                    start=False, stop=True,
                )
            nc.scalar.copy(
                vext_h[:, mt, :, 0:64], ps[:].rearrange("p (h c) -> p h c", c=64)
            )

        # ---- per-head attention
        apair = None
        for h in range(HPC):
            po = 64 * (h % 2)
            q_h = qkT_sb[po:po + 64, h // 2, :]
            k_h = qkT_sb[po:po + 64, 4 + h // 2, :]

            # scores (transposed): sT[j, i] = k_j . q_i ; p = m * exp(sT/8)
            pT = pT_pool.tile([P, JT, DL], BF16, tag="pT")
            for jt in range(JT):
                for nt in range(2):
                    ps = ps_mm.tile([P, 512], F32, tag="mm")
                    nc.tensor.matmul(
                        ps[:], k_h[:, bass.ts(jt, P)], q_h[:, bass.ts(nt, 512)],
                        start=True, stop=True,
                    )
                    nc.scalar.activation(
                        pT[:, jt, bass.ts(nt, 512)], ps[:], AF.Exp, scale=0.125
                    )
                nc.vector.tensor_mul(pT[:, jt, :], pT[:, jt, :], m_sb[:, jt, :])

            # SV_h = sum_j v_h[j, :]; SVdiv = SV_h / 1024 replicated to 128 rows
            pssv = ps_sv.tile([1, DH], F32, tag="sv")
            for jt in range(JT):
                nc.tensor.matmul(
                    pssv[:], ones_col[:, :], vext_h[:, jt, h, 0:64],
                    start=(jt == 0), stop=(jt == JT - 1),
                )
            nc.scalar.copy(svh_sb[0:1, h, :], pssv[:])
            pssvd = ps_svd.tile([P, DH], F32, tag="svd")
            nc.tensor.matmul(
                pssvd[:], inv_row[0:1, :], svh_sb[0:1, h, :], start=True, stop=True
            )

            # a[i, 0:64] unnormalized + denominator in col 64
            if h % 2 == 0:
                apair = apair_pool.tile([P, IT, P], BF16, tag="apair")
            for it in range(IT):
                psa = ps_pv.tile([P, 65], F32, tag="pv")
                for jt in range(JT):
                    nc.tensor.matmul(
                        psa[:], pT[:, jt, bass.ts(it, P)],
                        vext_h[:, jt, h, :],
                        start=(jt == 0), stop=(jt == JT - 1),
                    )
                flag = small_pool.tile([P, 1], mybir.dt.uint8, tag="flag")
                d2 = small_pool.tile([P, 1], F32, tag="d2")
                r = small_pool.tile([P, 1], F32, tag="r")
                nc.vector.tensor_scalar(
                    flag[:], psa[:, 64:65], 0.0, None, mybir.AluOpType.is_equal
                )
                # empty rows: numerator is exactly 0, so r's value is moot
                # there (the predicated copy below overwrites them).
                nc.vector.tensor_scalar(
                    d2[:], psa[:, 64:65], 1e-30, None, mybir.AluOpType.max
                )
                nc.vector.reciprocal(r[:], d2[:])
                dst = apair[:, it, po:po + 64]
                nc.vector.tensor_scalar(
                    dst, psa[:, 0:64], r[:], None, mybir.AluOpType.mult
                )
                nc.vector.copy_predicated(
                    dst, flag[:].to_broadcast((P, 64)), pssvd[:]
                )

            # transpose finished pair -> aT
            if h % 2 == 1:
                for it in range(IT):
                    pst = ps_tr.tile([P, P], BF16, tag="tr")
                    nc.tensor.transpose(pst[:], apair[:, it, :], ident[:])
                    nc.vector.tensor_copy(aT_sb[:, h // 2, bass.ts(it, P)], pst[:])

        # ---- out_partial[i, :] = aT.T @ Wp
        for it in range(IT):
            outst = outst_pool.tile([P, DL], F32, tag="outst")
            for nt in range(2):
                ps = ps_mm.tile([P, 512], F32, tag="mm")
                for kt in range(4):
                    nc.tensor.matmul(
                        ps[:], aT_sb[:, kt, bass.ts(it, P)],
                        wpj_sb[:, kt, bass.ts(nt, 512)],
                        start=(kt == 0), stop=(kt == 3),
                    )
                nc.scalar.copy(outst[:, bass.ts(nt, 512)], ps[:])
            nc.sync.dma_start(outs["outp"][bass.ts(it, P), :], outst[:])


def build_nc(use_bias):
    # Bacc (not plain Bass): its compile() runs move_matmul_waits_to_ldweights
    # and generate_event_semaphores, required to satisfy the 1-wait-per-
    # instruction codegen constraint on TRN2.
    nc = bacc.Bacc("TRN2", num_devices=8, name="sparse_attn")
    ins = {
        "xT": nc.dram_tensor("xT", (E, DL), BF16, kind="ExternalInput").ap(),
        "wqk": nc.dram_tensor("wqk", (E, 1024), BF16, kind="ExternalInput").ap(),
        "wv": nc.dram_tensor("wv", (E, 512), BF16, kind="ExternalInput").ap(),
        "wp": nc.dram_tensor("wp", (512, DL), BF16, kind="ExternalInput").ap(),
        "Ab": nc.dram_tensor("Ab", (DL, DL), BF16, kind="ExternalInput").ap(),
    }
    if use_bias:
        ins["baqk"] = nc.dram_tensor("baqk", (1, 1024), BF16,
                                     kind="ExternalInput").ap()
        ins["bav"] = nc.dram_tensor("bav", (1, 512), BF16,
                                    kind="ExternalInput").ap()
    outs = {
        "outp": nc.dram_tensor("outp", (DL, DL), F32, kind="ExternalOutput").ap(),
    }
    with tile.TileContext(nc) as tc:
        build_body(tc, ins, outs, use_bias)
    nc.compile()
    return nc


def prep_in_maps(inputs):
    x = np.asarray(inputs["x"], dtype=np.float32)
    R = np.asarray(inputs["rns_indices"]).astype(np.int64)
    Wa = np.asarray(inputs["W_attn"], dtype=np.float32)
    ba = np.asarray(inputs["b_attn"], dtype=np.float32)
    Wp = np.asarray(inputs["W_proj"], dtype=np.float32)

    # decode rns indices to the dense 0/1 selection matrix A[b,i,j]=[j in S_i]
    A = np.zeros((4, DL, DL), dtype=NPBF)
    A[np.arange(4)[:, None, None], np.arange(DL)[None, :, None], R] = 1.0

    use_bias = bool(np.any(ba != 0.0))
    in_maps = []
    for c in range(8):
        b, g = divmod(c, 2)
        qs, ks, vs = g * 512, 1024 + g * 512, 2048 + g * 512
        m = {
            "xT": np.ascontiguousarray(x[b].T).astype(NPBF),
            "wqk": np.ascontiguousarray(
                np.concatenate([Wa[:, qs:qs + 512], Wa[:, ks:ks + 512]], axis=1)
            ).astype(NPBF),
            "wv": np.ascontiguousarray(Wa[:, vs:vs + 512]).astype(NPBF),
            "wp": np.ascontiguousarray(Wp[g * 512:(g + 1) * 512, :]).astype(NPBF),
            "Ab": A[b],
        }
        if use_bias:
            m["baqk"] = np.concatenate(
                [ba[qs:qs + 512], ba[ks:ks + 512]]
            )[None, :].astype(NPBF)
            m["bav"] = np.ascontiguousarray(ba[vs:vs + 512][None, :]).astype(NPBF)
        in_maps.append(m)
    return in_maps, use_bias


def kernel(**inputs):
    global LAST_RESULT
    in_maps, use_bias = prep_in_maps(inputs)
    nc = build_nc(use_bias)
    trace = os.environ.get("KTRACE", "") == "1"
    if trace:
        try:  # the axon NTFF hook is absent in some containers
            from antenv.axon_hooks import get_axon_ntff_profile_hook  # noqa: F401
        except ImportError:
            trace = False
    res = run_bass_kernel_spmd(nc, in_maps, core_ids=list(range(8)), trace=trace)
    LAST_RESULT = res
    bp = np.asarray(inputs["b_proj"], dtype=np.float32)
    out = np.empty((4, DL, DL), dtype=np.float32)
    for b in range(4):
        out[b] = res.results[2 * b]["outp"] + res.results[2 * b + 1]["outp"] \
            + bp[None, :]
    return out
